# revision 19
# baseline (speedup 1.0000x reference)
"""Cumulative LayerNorm Trainium2 Bass kernel.

x: [B=8, C=256, T=16000] f32.  Per timestep t: normalize x[:, :, t] by the
mean/std of all elements x[:, :, t'<=t] (cumulative over channels+time), then
scale by weight[c] and add bias[c].

Sharding: pure data parallel over B across 8 NeuronCores (1 sample/core).

Per-core algorithm (C=256 = 2 halves of 128 partitions, T on the free dim):
  Phase A (per 2000-col io-tile):
    - One 3D DMA loads both channel halves into SBUF [128, 2, 2000]
      (labeled f32r so the PE consumes it directly; fp32r truncates operands
      to ~13 mantissa bits inside the PE only).
    - xx = x^2 in bf16 (ACT for half 0, GPSIMD for half 1).
    - PE: s[t] = sum_c x (fp32r ones weights) and sq[t] = sum_c x^2 (bf16)
      as [2, 2, 512] PSUM row-blocks per 1000-col group; ACT evacuates row 0
      to a [1, 1000] SBUF row; a reshape DMA (issued from GPSIMD, cheap
      there) scatters it into the [128, 125] "stat layout" where t = 125p+i.
  Stats (per 4000-col chunk = 32 stat rows; engine ops need 32-aligned
  partition bases):
    - DVE tensor_tensor_scan along i (per-partition prefix sums).
    - Row totals go to st[128, 2]; a strict-upper-triangular fp32r matmul
      gives exclusive cross-partition offsets (future rows zeroed, so one
      full-K matmul per chunk is exact).
    - nm = -(scan + off) * invcnt directly (negation folded into the
      constant); var = E[x^2] - nm^2; istd = 1/sqrt(var + eps) (ACT Sqrt +
      DVE reciprocal).  nm and istd land in one [128, 2, 125] surface.
  Phase C (per io-tile):
    - ONE gather DMA (issued from ACT right after the chunk chain) pulls the
      tile's 16 stat rows into a row buffer brow[1, 16, 2, 125].
    - Per 1000-col half-tile: PE rank-1 broadcasts nm_bc and ibc into PSUM.
    - DVE scalar_tensor_tensor pair per channel half: z = nm_bc + x;
      y = (z * w[p]) * ibc -- the second STT reads ibc straight from PSUM
      (no ACT copy).  y lands in a per-(tile, half) [128, 2000] staging tile
      stored with one DMA.

Emission is software-pipelined at io-tile granularity (phase C lags the
x-loads by 4 tiles) and each engine's FIFO is ordered so streaming work never
queues behind the long-latency stats chain.
"""
import ml_dtypes
import numpy as np

B, C, T = 8, 256, 16000
P = 128
NH = 2                     # channel halves
CHUNK = 2000               # t per io-tile
NCHUNK = T // CHUNK        # 8
ROWS = T // P              # 125  (stat layout free dim; t = 125*p + i)
PB = 500                   # psum block columns (4 per io-tile)
LAG = 4                    # x-load leads phase C by this many tiles
EPS = 1e-06

_cached = {}


def _build_nc(with_bias: bool):
    from contextlib import ExitStack

    import concourse.tile as tile
    from concourse import bacc, mybir

    f32 = mybir.dt.float32
    f32r = mybir.dt.float32r
    bf16 = mybir.dt.bfloat16
    ALU = mybir.AluOpType
    ACTF = mybir.ActivationFunctionType

    nc = bacc.Bacc()

    x = nc.dram_tensor("x", [C, T], f32, kind="ExternalInput")
    wvec = nc.dram_tensor("wvec", [C, 1], f32, kind="ExternalInput")
    tri_d = nc.dram_tensor("tri", [P, P], f32r, kind="ExternalInput")
    ones2r_d = nc.dram_tensor("ones2r", [P, 2], f32r, kind="ExternalInput")
    ones2b_d = nc.dram_tensor("ones2b", [P, 2], bf16, kind="ExternalInput")
    onesb_d = nc.dram_tensor("onesb", [1, P], f32r, kind="ExternalInput")
    onesbb_d = nc.dram_tensor("onesbb", [1, P], bf16, kind="ExternalInput")
    zeros2_d = nc.dram_tensor("zeros2", [P, 2], f32r, kind="ExternalInput")
    invcnt_d = nc.dram_tensor("invcnt", [P, ROWS], f32, kind="ExternalInput")
    ninvcnt_d = nc.dram_tensor("ninvcnt", [P, ROWS], f32, kind="ExternalInput")
    if with_bias:
        bvec = nc.dram_tensor("bvec", [C, 1], f32, kind="ExternalInput")
    y = nc.dram_tensor("y", [C, T], f32, kind="ExternalOutput")

    with tile.TileContext(nc) as tc, ExitStack() as ctx:
        const = ctx.enter_context(tc.tile_pool(name="const", bufs=1))
        persist = ctx.enter_context(tc.tile_pool(name="persist", bufs=1))
        xpool = ctx.enter_context(tc.tile_pool(name="xpool", bufs=6))
        ypool = ctx.enter_context(tc.tile_pool(name="ypool", bufs=2))
        sqpool = ctx.enter_context(tc.tile_pool(name="sqpool", bufs=2))
        erow = ctx.enter_context(tc.tile_pool(name="erow", bufs=3))
        brpool = ctx.enter_context(tc.tile_pool(name="brow", bufs=4))
        ps_s = ctx.enter_context(tc.tile_pool(name="ps_s", bufs=2, space="PSUM"))
        ps_nm = ctx.enter_context(tc.tile_pool(name="ps_nm", bufs=2, space="PSUM"))
        ps_i = ctx.enter_context(tc.tile_pool(name="ps_i", bufs=2, space="PSUM"))
        zpool = ctx.enter_context(tc.tile_pool(name="zpool", bufs=3))

        # ---- constants (issued from DVE: it is idle all warmup, and this
        # keeps the Sync queue free so x-loads trigger immediately) ----
        tri = const.tile([P, P], f32r)
        nc.gpsimd.dma_start(out=tri, in_=tri_d[:, :])
        ones2r = const.tile([P, 2], f32r)
        nc.gpsimd.dma_start(out=ones2r, in_=ones2r_d[:, :])
        ones2b = const.tile([P, 2], bf16)
        nc.gpsimd.dma_start(out=ones2b, in_=ones2b_d[:, :])
        onesb = const.tile([1, P], f32r)
        nc.gpsimd.dma_start(out=onesb, in_=onesb_d[:, :])
        onesbb = const.tile([1, P], bf16)
        nc.gpsimd.dma_start(out=onesbb, in_=onesbb_d[:, :])
        invcnt = const.tile([P, ROWS], f32)
        nc.gpsimd.dma_start(out=invcnt, in_=invcnt_d[:, :])
        ninvcnt = const.tile([P, ROWS], f32)
        nc.gpsimd.dma_start(out=ninvcnt, in_=ninvcnt_d[:, :])
        w_sb = const.tile([P, NH], f32)
        for h in range(NH):
            nc.gpsimd.dma_start(out=w_sb[:, h : h + 1], in_=wvec[h * P : (h + 1) * P, 0:1])
        if with_bias:
            b_sb = const.tile([P, NH], f32)
            for h in range(NH):
                nc.gpsimd.dma_start(
                    out=b_sb[:, h : h + 1], in_=bvec[h * P : (h + 1) * P, 0:1]
                )
        eps_sb = const.tile([P, 1], f32)
        nc.vector.memset(eps_sb, EPS)

        # ---- persistent stat-layout surfaces ----
        s_re = persist.tile([P, ROWS], f32)     # channel sums -> prefix sums
        sq_re = persist.tile([P, ROWS], f32)
        nmist = persist.tile([P, 2, ROWS], bf16)  # plane 0: -mean, plane 1: istd
        ex2_t = persist.tile([P, ROWS], f32)    # E[x^2] -> var
        msq_t = persist.tile([P, ROWS], f32)    # mean^2 -> sqrt(var+eps)
        st_sb = persist.tile([P, 2], f32r)      # chunk totals (s, sq)
        nc.gpsimd.dma_start(out=st_sb, in_=zeros2_d[:, :])

        tiles = {}

        def load_x(tix, eng=None):
            """One 3D DMA for both halves of io-tile tix (issued from SP; the
            warmup's tile 1 goes out on the ACT queue so the two first loads
            stream on separate hardware DMA queues)."""
            t0 = tix * CHUNK
            x_t = xpool.tile([P, NH, CHUNK], f32r, tag="x", name="x_t")
            (eng or nc.sync).dma_start(
                out=x_t,
                in_=x.rearrange("(h p) t -> p h t", h=NH)[
                    :, :, t0 : t0 + CHUNK
                ].bitcast(f32r),
            )
            tiles[tix] = x_t

        sq_tiles = {}

        def squares(tix, act_both=False):
            """x^2 in bf16: ACT for half 0, GPSIMD for half 1 (or both on ACT
            during warmup, when GPSIMD serializing would delay the chain)."""
            x_t = tiles[tix]
            xx0 = sqpool.tile([P, CHUNK], bf16, tag="xx0", name="xx0")
            nc.scalar.activation(xx0, x_t[:, 0, :].bitcast(f32), ACTF.Square)
            xx1 = sqpool.tile([P, CHUNK], bf16, tag="xx1", name="xx1")
            if act_both:
                nc.scalar.activation(xx1, x_t[:, 1, :].bitcast(f32), ACTF.Square)
            else:
                nc.gpsimd.tensor_tensor(
                    xx1, x_t[:, 1, :].bitcast(f32), x_t[:, 1, :].bitcast(f32),
                    ALU.mult,
                )
            sq_tiles[tix] = (xx0, xx1)

        def a_sums_s(tix, dve_evac=False):
            """Channel sums of x -> stat rows (PE mms, ACT/DVE evac, Pool DMA).
            dve_evac routes the PSUM evacuation to the (warmup-idle) DVE."""
            x_t = tiles[tix]
            for a2 in range(2):  # 1000-col groups
                sps = ps_s.tile([2, 2, 512], f32, tag="stat", name="sps")
                for j in range(2):
                    cs = slice((2 * a2 + j) * PB, (2 * a2 + j + 1) * PB)
                    nc.tensor.matmul(
                        sps[0:2, j, 0:PB], ones2r, x_t[:, 0, cs],
                        start=True, stop=False,
                    )
                    nc.tensor.matmul(
                        sps[0:2, j, 0:PB], ones2r, x_t[:, 1, cs],
                        start=False, stop=True,
                    )
                srow = erow.tile([1, 1024], f32, tag="erow", name="srow")
                sr_ap = srow[0:1, 0:1000].rearrange("p (j n) -> p j n", j=2)
                if dve_evac:
                    nc.vector.tensor_copy(sr_ap, sps[0:1, :, 0:PB])
                else:
                    nc.scalar.copy(sr_ap, sps[0:1, :, 0:PB])
                rp = 16 * tix + 8 * a2
                nc.gpsimd.dma_start(out=s_re[rp : rp + 8, :], in_=srow[0:1, 0:1000])

        def a_sums_q(tix, dve_evac=False):
            """Channel sums of x^2 -> stat rows."""
            xx0, xx1 = sq_tiles.pop(tix)
            for a2 in range(2):
                qps = ps_s.tile([2, 2, 512], f32, tag="stat", name="qps")
                for j in range(2):
                    cs = slice((2 * a2 + j) * PB, (2 * a2 + j + 1) * PB)
                    nc.tensor.matmul(
                        qps[0:2, j, 0:PB], ones2b, xx0[:, cs], start=True, stop=False
                    )
                    nc.tensor.matmul(
                        qps[0:2, j, 0:PB], ones2b, xx1[:, cs], start=False, stop=True
                    )
                qrow = erow.tile([1, 1024], f32, tag="erow", name="qrow")
                qr_ap = qrow[0:1, 0:1000].rearrange("p (j n) -> p j n", j=2)
                if dve_evac:
                    nc.vector.tensor_copy(qr_ap, qps[0:1, :, 0:PB])
                else:
                    nc.scalar.copy(qr_ap, qps[0:1, :, 0:PB])
                rp = 16 * tix + 8 * a2
                nc.gpsimd.dma_start(out=sq_re[rp : rp + 8, :], in_=qrow[0:1, 0:1000])

        def stats_scan(sc):
            """Prefix sums + chunk totals for stat rows 32*sc .. 32*sc+32."""
            sl = slice(32 * sc, 32 * sc + 32)
            nc.vector.tensor_tensor_scan(
                out=s_re[sl, :], data0=s_re[sl, :], data1=s_re[sl, :],
                initial=0.0, op0=ALU.add, op1=ALU.bypass,
            )
            nc.vector.tensor_tensor_scan(
                out=sq_re[sl, :], data0=sq_re[sl, :], data1=sq_re[sl, :],
                initial=0.0, op0=ALU.add, op1=ALU.bypass,
            )
            nc.vector.tensor_copy(st_sb[sl, 0:1], s_re[sl, ROWS - 1 : ROWS])
            nc.vector.tensor_copy(st_sb[sl, 1:2], sq_re[sl, ROWS - 1 : ROWS])

        def stats_fin(sc):
            """Offsets + nm/istd for stat rows 32*sc .. 32*sc+32."""
            sl = slice(32 * sc, 32 * sc + 32)
            offps = ps_s.tile([P, 2], f32, tag="stat", name="offps")
            nc.tensor.matmul(offps, tri, st_sb, start=True, stop=True)

            # nm = -(s + off) / cnt  (negation folded into the constant).
            # bf16 output: |mean| error ~0.4% rel, far inside the 2e-2 gate.
            with nc.allow_low_precision(reason="bf16 nm/istd broadcast rows"):
                nc.vector.scalar_tensor_tensor(
                    out=nmist[sl, 0, :], in0=s_re[sl, :], scalar=offps[sl, 0:1],
                    in1=ninvcnt[sl, :], op0=ALU.add, op1=ALU.mult,
                )
            nc.vector.scalar_tensor_tensor(
                out=ex2_t[sl, :], in0=sq_re[sl, :], scalar=offps[sl, 1:2],
                in1=invcnt[sl, :], op0=ALU.add, op1=ALU.mult,
            )
            nc.vector.tensor_tensor(
                msq_t[sl, :], nmist[sl, 0, :], nmist[sl, 0, :], ALU.mult
            )
            nc.vector.tensor_tensor(ex2_t[sl, :], ex2_t[sl, :], msq_t[sl, :], ALU.subtract)
            # istd = 1 / sqrt(var + eps)  (Sqrt keeps the ACT table set stable)
            nc.scalar.activation(
                msq_t[sl, :], ex2_t[sl, :], ACTF.Sqrt, bias=eps_sb[sl, :], scale=1.0
            )
            with nc.allow_low_precision(reason="bf16 nm/istd broadcast rows"):
                nc.vector.reciprocal(out=nmist[sl, 1, :], in_=msq_t[sl, :])

        def gather(tix):
            """Two DMAs: the tile's 16 nm/istd stat rows -> brow planes
            [1, 2, 16, 125] (plane-major so broadcast rhs slices stay
            contiguous).  Issued from ACT, right behind the chunk's Sqrt."""
            rsl = slice(16 * tix, 16 * tix + 16)
            brow = brpool.tile([1, 2, 16, ROWS], bf16, tag="brow", name="brow")
            nc.scalar.dma_start(out=brow[:, 0, :, :], in_=nmist[rsl, 0, :])
            nc.scalar.dma_start(out=brow[:, 1, :, :], in_=nmist[rsl, 1, :])
            return brow

        def bc_mms(tix, brow, blk):
            """PE rank-1 broadcasts for 500-col block `blk` (0..3)."""
            nm_ps = ps_nm.tile([P, 512], f32, tag="nm", name="nm_ps")
            ibc = ps_i.tile([P, 512], f32, tag="ibc_ps", name="ibc")
            r0 = 4 * blk
            nc.tensor.matmul(
                nm_ps[:, 0:PB], onesbb, brow[0:1, 0, r0 : r0 + 4, :],
                start=True, stop=True,
            )
            nc.tensor.matmul(
                ibc[:, 0:PB], onesbb, brow[0:1, 1, r0 : r0 + 4, :],
                start=True, stop=True,
            )
            return nm_ps, ibc

        def applies(tix, blk, nm_ps, ibc, y_st):
            """DVE applies for 500-col block `blk`: one z op covering both
            channel halves (nm broadcast over h via a stride-0 AP), then a
            y op per half (w is per-(partition, half), so it must stay a
            per-half scalar)."""
            from concourse.bass import AP as BassAP

            x_t = tiles[tix]
            cs = slice(blk * PB, (blk + 1) * PB)
            nm_a = nm_ps[:, 0:PB]
            nm_bc = BassAP(nm_a.tensor, nm_a.offset,
                           [nm_a.ap[0], [0, NH], nm_a.ap[1]])
            z_sb = zpool.tile([P, NH, PB], f32, tag="z", name="z_sb")
            nc.vector.scalar_tensor_tensor(
                out=z_sb, in0=nm_bc, scalar=1.0,
                in1=x_t[:, :, cs].bitcast(f32), op0=ALU.mult, op1=ALU.add,
            )
            for h in range(NH):
                # y = (z * w) * istd, istd read straight from PSUM
                y_ap = y_st[h][:, cs]
                nc.vector.scalar_tensor_tensor(
                    out=y_ap, in0=z_sb[:, h, :], scalar=w_sb[:, h : h + 1],
                    in1=ibc[:, 0:PB], op0=ALU.mult, op1=ALU.mult,
                )
                if with_bias:
                    nc.vector.tensor_scalar_add(
                        out=y_ap, in0=y_ap, scalar1=b_sb[:, h : h + 1]
                    )

        # ---- software-pipelined emission ----
        # x-loads lead phase C by 4 tiles, A-compute by 3, and the stats
        # chain + gather for chunk sc run TWO rounds before C consumes them,
        # so phase C's broadcasts never wait on the serial chain.  Per-engine
        # FIFO orders per round k:
        #   SP:   y-stores(k-1), x-load(k+4)
        #   PE:   nmA(k), ibcA(k), A-mms(k+3), tri(sc), nmB(k), ibcB(k)
        #   ACT:  xx0(k+3), evacs(k+3), sqrt(sc), gathers(k+2, k+3)
        #   DVE:  applies halfA(k), chain(sc), applies halfB(k)
        #   Pool: xx1(k+3), stat DMAs(k+3)
        brows = {}
        ystore = {}

        def emit_stores(k):
            t0 = k * CHUNK
            y_st = ystore.pop(k)
            for h in range(NH):
                nc.sync.dma_start(
                    out=y[h * P : (h + 1) * P, t0 : t0 + CHUNK], in_=y_st[h]
                )

        # Prologue: loads first so x(0) streams immediately; tile 1 loads on
        # the ACT DMA queue (parallel hw queue); tiles 0-1 square on ACT and
        # evacuate on the idle DVE so the first chain closes fast.
        for t in range(5):
            load_x(t)
        squares(0, act_both=True)
        a_sums_s(0)
        squares(1, act_both=True)
        a_sums_s(1)
        a_sums_q(0)
        a_sums_q(1)
        stats_scan(0)
        stats_fin(0)
        squares(2)
        a_sums_s(2)
        a_sums_q(2)
        brows[0] = gather(0)
        brows[1] = gather(1)
        squares(3)
        a_sums_s(3)
        a_sums_q(3)

        for k in range(NCHUNK):
            if k >= 1:
                emit_stores(k - 1)
            if k + 5 < NCHUNK:
                load_x(k + 5)
            y_st = {
                h: ypool.tile([P, CHUNK], f32, tag=f"y{h}", name=f"y{h}")
                for h in range(NH)
            }
            ystore[k] = y_st
            brow = brows.pop(k)
            if k + 4 < NCHUNK:
                squares(k + 4)
            bc0 = bc_mms(k, brow, 0)
            bc1 = bc_mms(k, brow, 1)
            applies(k, 0, *bc0, y_st)
            if k + 4 < NCHUNK:
                a_sums_s(k + 4)
            applies(k, 1, *bc1, y_st)
            bc2 = bc_mms(k, brow, 2)
            if k + 4 < NCHUNK:
                a_sums_q(k + 4)
            if k % 2 == 0 and k + 2 < NCHUNK:
                stats_scan((k + 2) // 2)
                stats_fin((k + 2) // 2)
            applies(k, 2, *bc2, y_st)
            bc3 = bc_mms(k, brow, 3)
            applies(k, 3, *bc3, y_st)
            if k % 2 == 0 and k + 2 < NCHUNK:
                brows[k + 2] = gather(k + 2)
                brows[k + 3] = gather(k + 3)
            tiles.pop(k)
        emit_stores(NCHUNK - 1)
    nc.compile()
    return nc


def _consts():
    tri = np.triu(np.ones((P, P), dtype=np.float32), k=1)  # tri[k,m]=1 iff k<m
    ones2 = np.ones((P, 2), dtype=np.float32)
    onesb = np.ones((1, P), dtype=np.float32)
    t_idx = (125 * np.arange(P, dtype=np.float64)[:, None]
             + np.arange(ROWS, dtype=np.float64)[None, :])
    invcnt = (1.0 / (C * (t_idx + 1.0))).astype(np.float32)
    return {"tri": tri, "ones2r": ones2,
            "ones2b": ones2.astype(ml_dtypes.bfloat16), "onesb": onesb,
            "onesbb": onesb.astype(ml_dtypes.bfloat16),
            "zeros2": np.zeros((P, 2), dtype=np.float32),
            "invcnt": invcnt, "ninvcnt": -invcnt}


def _get_nc(with_bias: bool):
    key = ("nc", with_bias)
    if key not in _cached:
        _cached[key] = _build_nc(with_bias)
    return _cached[key]


def _run(x, weight, bias, trace=False):
    from concourse.bass_utils import run_bass_kernel_spmd

    x = np.ascontiguousarray(np.asarray(x, dtype=np.float32))
    weight = np.asarray(weight, dtype=np.float32).reshape(C, 1)
    bias = np.asarray(bias, dtype=np.float32).reshape(C, 1)
    with_bias = bool(np.any(bias))
    nc = _get_nc(with_bias)

    consts = _consts()
    in_maps = []
    for b in range(B):
        m = {"x": np.ascontiguousarray(x[b]), "wvec": weight}
        if with_bias:
            m["bvec"] = bias
        m.update(consts)
        in_maps.append(m)

    res = run_bass_kernel_spmd(nc, in_maps, core_ids=list(range(B)), trace=trace)
    y = np.stack([r["y"] for r in res.results], axis=0)
    return y, res


def kernel(x, weight, bias):
    y, _ = _run(x, weight, bias, trace=False)
    return y
